# revision 29
# baseline (speedup 1.0000x reference)
"""Trainium2 Bass kernel for CapsuleLayer (dynamic routing), 8-core data-parallel.

Problem: x [128, 1152, 512] f32, W [512, 160] f32.
  u_hat = (x @ W).reshape(B, N, 10, 16)
  b = 0; 3 routing iterations of softmax/weighted-sum/squash.
Output: v [128, 10, 16] f32.

Sharding: data-parallel over batch. Each of the 8 cores gets 16 batches
(x shard [16*1152, 512]) and the full W; no cross-core communication.

Per-core pipeline:
  Phase 1 (streamed over 144 row-tiles of 128):
    - DMA x tile f32 -> SBUF, cast to bf16 (DVE or SWDGE cast-DMA)
    - transpose each [128,128] block on the PE as a *regular* bf16 matmul
      against an identity moving operand (full speed, unlike transpose-mode)
    - u_hat tile = xT.T @ W accumulated over the 4 k-chunks in PSUM,
      copied to SBUF as bf16 in [n, 160] layout (natural for routing)
  Phase 2 (on-chip routing, u_hat resident in SBUF):
    - capsule sums s: wide DVE tree-adds over the 9 tiles of each batch,
      then a single ones-column matmul for the 128-partition reduction
    - softmax over capsules without max-subtraction (|b| stays small)
    - squash per batch on [1,160] tiles; b update via one whole-core
      multiply + grouped reduce
"""

import os
import sys

import numpy as np

sys.path.insert(0, "/opt/trn_rl_repo")

import concourse.bass as bass
import concourse.tile as tile
import concourse.mybir as mybir
from concourse import bacc

F32 = mybir.dt.float32
BF16 = mybir.dt.bfloat16

B, N, K, C, D = 128, 1152, 512, 10, 16
CD = C * D  # 160
NCORES = 8
BSH = B // NCORES  # batches per core

XB_BATCH = 4  # n-tiles per x-load DMA (steady state)
GP_ROUTING = True  # offload f32 (1x-anyway) routing elementwise work to gpsimd
RSQRT_NEWTON = 1  # >0: squash sqrt via quake-rsqrt, entirely on DVE (tiny ops);
#                   keeps ACT on the exp table set all kernel (no table reloads)
# greedy psum->sbuf copy balancing: estimated ns per copy kind per engine.
# gpsimd cannot access PSUM, so only v (DVE) and s (ACT) are eligible.
# DVE reads bf16-PSUM at 2x, ACT only at 1x; both are 1x on f32-PSUM.
XT2_COST = {"v": 658, "s": 996}  # paired 2-tile bf16 xT copy (1024 elem)
XT1_COST = {"v": 392, "s": 570}  # singleton xT copy at a group boundary
UH_COST = {"v": 625, "s": 585}
SA_COST = {"v": 292, "s": 318}


def build_core_program(bsh=BSH, n_len=N, nc=None):
    """Build the single-core Bass program for a shard of `bsh` batches."""
    assert n_len % 128 == 0
    tpb = n_len // 128  # row-tiles per batch
    nt = bsh * tpb  # total row-tiles
    rows = bsh * n_len

    if nc is None:
        nc = bacc.Bacc("TRN2", target_bir_lowering=False, debug=False)

    x_in = nc.declare_dram_parameter("x", [rows, K], F32, isOutput=False).ap()
    w_in = nc.declare_dram_parameter("W", [K, CD], F32, isOutput=False).ap()
    id_in = nc.declare_dram_parameter("ident", [128, 128], BF16, isOutput=False).ap()
    mk_in = nc.declare_dram_parameter("mask", [C, CD], BF16, isOutput=False).ap()
    v_out = nc.declare_dram_parameter("v", [bsh, CD], F32, isOutput=True).ap()

    with tile.TileContext(nc) as tc:
        _build_body(tc, x_in, w_in, id_in, mk_in, v_out, bsh, tpb)
    nc.finalize()
    return nc


def _build_body(tc, x_in, w_in, id_in, mk_in, v_out, bsh, tpb):
    nc = tc.nc
    nt = bsh * tpb
    KT = K // 128  # 4 contraction chunks

    from contextlib import ExitStack

    with ExitStack() as ctx:
        singles = ctx.enter_context(tc.tile_pool(name="singles", bufs=1))
        persist = ctx.enter_context(tc.tile_pool(name="persist", bufs=1))
        pool_xb = ctx.enter_context(tc.tile_pool(name="xb", bufs=4))
        pool_xT = ctx.enter_context(tc.tile_pool(name="xT", bufs=8))
        pool_sm = ctx.enter_context(tc.tile_pool(name="smalls", bufs=6))
        ps_U = ctx.enter_context(tc.tile_pool(name="psU", bufs=2, space="PSUM"))
        ps_P = ctx.enter_context(tc.tile_pool(name="psP", bufs=2, space="PSUM"))
        ps_S = ctx.enter_context(tc.tile_pool(name="psS", bufs=2, space="PSUM"))
        ps_T = ctx.enter_context(tc.tile_pool(name="psT", bufs=2, space="PSUM"))

        # --- constants ---
        ident = singles.tile([128, 128], BF16)
        nc.sync.dma_start(out=ident, in_=id_in)
        mask = singles.tile([C, CD], BF16)
        nc.sync.dma_start(out=mask, in_=mk_in)
        # all-ones / all-0.1 stationary operands: the capsule-sum matmuls use
        # M=128 so the (identical) column sums land replicated on all
        # partitions, which lets squash and the b-update run without any
        # partition-broadcast (illegal on DVE).
        ones_m = singles.tile([128, 128], BF16)
        nc.vector.memset(ones_m, 1.0)
        tenth_m = singles.tile([128, 128], BF16)
        nc.vector.memset(tenth_m, 1.0 / C)

        w_f32 = singles.tile([128, KT, CD], F32)
        nc.sync.dma_start(out=w_f32, in_=w_in.rearrange("(j p) c -> p j c", p=128))
        w_bf = singles.tile([128, KT, CD], BF16)
        nc.vector.tensor_copy(w_bf, w_f32)

        # --- persistent tensors ---
        u_hat = persist.tile([128, nt, CD], BF16)
        w_scr = persist.tile([128, nt, CD], BF16)
        b_log = persist.tile([128, nt * C], F32)
        e_exp = persist.tile([128, nt * C], F32)
        c_sm = persist.tile([128, nt * C], BF16)
        ssum = persist.tile([128, nt], F32)
        s_all = persist.tile([128, bsh, CD], F32)
        sq_all = persist.tile([128, bsh, CD], F32)
        n2_all = persist.tile([128, bsh * C], F32)
        fc_all = persist.tile([128, bsh * C], F32)
        y_all = persist.tile([128, bsh * C], F32)
        t_all = persist.tile([128, bsh * C], F32)
        vrep_bf = persist.tile([128, bsh, CD], BF16)
        vrep = persist.tile([128, bsh, CD], F32)

        # views
        u4 = u_hat[:].rearrange("p (g t) c -> p g t c", g=bsh)
        w4 = w_scr[:].rearrange("p (g t) c -> p g t c", g=bsh)
        w5 = w_scr[:].rearrange("p t (c d) -> p t c d", d=D)
        b3 = b_log[:].rearrange("p (t c) -> p t c", c=C)

        # ---------------- Phase 1 emitters ----------------
        # Transposes run on the PE in transpose-mode (bf16 out in PSUM);
        # TWO tiles share one PSUM bank and get ONE int32-bitcast copy
        # (halves ACT element count — ACT gets no 2x accel on bf16-PSUM —
        # and halves the fixed per-op cost).  The GEMMs of pair m-1 are
        # emitted after the transposes of pair m so the PE never stalls on
        # the copy.
        I32 = mybir.dt.int32
        # x-load batches: tiny first batches so the PE starts early.
        XBATCHES = []
        _t = 0
        _sizes = [2, 2] + [XB_BATCH] * nt
        for sz in _sizes:
            if _t >= nt:
                break
            sz = min(sz, nt - _t)
            XBATCHES.append((_t, sz))
            _t += sz
        TILE2XB = {}
        for _bi, (_ts0, _tn) in enumerate(XBATCHES):
            for _off in range(_tn):
                TILE2XB[_ts0 + _off] = (_bi, _off)

        xb_cur = [None]
        pu_cur = [None]
        copy_eng = {
            "v": nc.vector.tensor_copy,
            "s": nc.scalar.copy,
            "g": nc.gpsimd.tensor_copy,
        }
        # running per-engine busy-ns estimates; phase-2 emitters call track()
        # so copy placement adapts to where the schedule actually is.
        load = {"v": 0.0, "s": 0.0}

        def track(e, ns):
            load[e] += ns

        def pick_copy_eng(cost):
            e = min("vs", key=lambda k: load[k] + cost[k])
            load[e] += cost[e]
            return copy_eng[e]

        pair_state = {"pt8": None, "tiles": []}

        def flush_pair():
            pt8 = pair_state["pt8"]
            tiles = pair_state["tiles"]
            if pt8 is None:
                return []
            xt8 = pool_xT.tile([128, 2, KT, 128], BF16, tag="xt8")
            if len(tiles) == 2:
                pick_copy_eng(XT2_COST)(xt8, pt8)
            else:
                pick_copy_eng(XT1_COST)(xt8[:, 0:1], pt8[:, 0:1])
            out = [(t, xt8[:, i]) for i, t in enumerate(tiles)]
            pair_state["pt8"] = None
            pair_state["tiles"] = []
            return out

        def emit_trans(t):
            tb, tt = TILE2XB[t]
            if tt == 0:
                ts0, tn = XBATCHES[tb]
                xb = pool_xb.tile([128, tn, K], BF16, tag=f"xb{tn}")
                src = x_in[ts0 * 128 : (ts0 + tn) * 128, :]
                nc.gpsimd.dma_start(
                    out=xb, in_=src.rearrange("(t p) k -> p t k", p=128)
                )
                xb_cur[0] = xb
            xb = xb_cur[0]
            if pair_state["pt8"] is None:
                pair_state["pt8"] = ps_T.tile([128, 2, KT, 128], BF16, tag="psT8", name="pt8")
            pt8 = pair_state["pt8"]
            sl = len(pair_state["tiles"])
            for j in range(KT):
                nc.tensor.matmul(
                    pt8[:, sl, j, :],
                    lhsT=xb[:, tt, j * 128 : (j + 1) * 128],
                    rhs=ident,
                    start=True,
                    stop=True,
                    is_transpose=True,
                )
            pair_state["tiles"].append(t)
            if len(pair_state["tiles"]) == 2:
                return flush_pair()
            return []

        UCP = 3 if tpb % 3 == 0 else (2 if tpb % 2 == 0 else 1)  # tiles per u-copy

        def emit_gemm(t, xt4):
            lt = t % tpb
            loc = lt % UCP
            if loc == 0 or pu_cur[0] is None:
                pu_cur[0] = ps_U.tile([128, UCP, CD], F32, tag="psU2", name="pu2")
            pu2 = pu_cur[0]
            for j in range(KT):
                nc.tensor.matmul(
                    pu2[:, loc, :],
                    lhsT=xt4[:, j, :],
                    rhs=w_bf[:, j, :],
                    start=(j == 0),
                    stop=(j == KT - 1),
                )
            if loc == UCP - 1:
                pick_copy_eng(UH_COST)(u_hat[:, t - UCP + 1 : t + 1, :], pu2)
                pu_cur[0] = None
            elif lt == tpb - 1:
                pick_copy_eng(UH_COST)(
                    u_hat[:, t - loc : t + 1, :], pu2[:, 0 : loc + 1, :]
                )
                pu_cur[0] = None

        # ---------------- Phase 2 (routing) emitters, per group ----------
        # Non-uniform groups: big groups early (throughput — fixed per-op
        # costs amortize while phase-1 hides them), small groups last (the
        # post-phase-1 tail is latency-bound on the serial per-group chain,
        # which scales with group size).
        if bsh == 16:
            GSIZES = [3, 3, 2, 2, 2, 2, 1, 1]
        else:
            _ng = 8 if bsh % 8 == 0 else (4 if bsh % 4 == 0 else bsh)
            GSIZES = [bsh // _ng] * _ng
        GROUPS = []
        _o = 0
        for _s in GSIZES:
            GROUPS.append((_o, _s))
            _o += _s
        assert _o == bsh
        NG = len(GROUPS)

        def squash_group(gr, last):
            # v = s * sqrt(n2)/(1+n2)  (the +1e-7 of the reference only
            # perturbs v by O(1e-7) absolute, so it is dropped)
            g0, gb = GROUPS[gr]
            gs = slice(g0, g0 + gb)
            cs = slice(g0 * C, (g0 + gb) * C)
            s_g = s_all[:, gs, :]
            sq_g = sq_all[:, gs, :]
            n2_g = n2_all[:, cs]
            fc_g = fc_all[:, cs]
            track("s", 450)
            nc.scalar.square(sq_g, s_g)
            nc.vector.tensor_reduce(
                n2_g,
                sq_g.rearrange("p g (c d) -> p (g c) d", d=D),
                axis=mybir.AxisListType.X,
                op=mybir.AluOpType.add,
            )
            nc.vector.tensor_scalar_add(fc_g, n2_g, 1.0)
            nc.vector.reciprocal(fc_g, fc_g)
            if RSQRT_NEWTON > 0:
                # quake rsqrt: DVE int seed, gpsimd newton.  Keeps the ACT
                # engine on the exp table set for the whole kernel.
                I32 = mybir.dt.int32
                y_g = y_all[:, cs]
                t_g = t_all[:, cs]
                nc.vector.tensor_scalar(
                    out=y_g.bitcast(I32),
                    in0=n2_g.bitcast(I32),
                    scalar1=1,
                    scalar2=None,
                    op0=mybir.AluOpType.logical_shift_right,
                )
                nc.vector.tensor_scalar(
                    out=y_g.bitcast(I32),
                    in0=y_g.bitcast(I32),
                    scalar1=-1,
                    scalar2=0x5F3759DF,
                    op0=mybir.AluOpType.mult,
                    op1=mybir.AluOpType.add,
                )
                for _ in range(RSQRT_NEWTON):
                    nc.vector.tensor_mul(t_g, y_g, y_g)
                    nc.vector.tensor_mul(t_g, t_g, n2_g)
                    nc.vector.tensor_scalar(
                        out=t_g,
                        in0=t_g,
                        scalar1=-0.5,
                        scalar2=1.5,
                        op0=mybir.AluOpType.mult,
                        op1=mybir.AluOpType.add,
                    )
                    nc.vector.tensor_mul(y_g, y_g, t_g)
                nc.vector.tensor_mul(y_g, y_g, n2_g)  # sqrt(n2) = n2*rsqrt
                nc.vector.tensor_mul(fc_g, fc_g, y_g)  # sqrt(n2)/(1+n2)
            else:
                nc.scalar.sqrt(n2_g, n2_g)
                nc.vector.tensor_mul(fc_g, fc_g, n2_g)  # sqrt(n2)/(1+n2)
            track("v", 800 * gb)
            fb = fc_g.broadcast_to([128, gb * C, D])
            out = (vrep if last else vrep_bf)[:, gs, :]
            out_eng = nc.gpsimd if GP_ROUTING else nc.vector
            out_eng.tensor_mul(
                out.rearrange("p g (c d) -> p (g c) d", d=D),
                s_g.rearrange("p g (c d) -> p (g c) d", d=D),
                fb,
            )
            if last:
                nc.sync.dma_start(
                    out=v_out[g0 : g0 + gb, :], in_=vrep[0:1, gs, :]
                )

        def rt0(gr):
            # s0 = 0.1 * sum_{n,t} u_hat per batch, on the PE
            g0, gb = GROUPS[gr]
            for g in range(g0, g0 + gb):
                sp = ps_S.tile([128, CD], F32, tag="psS")
                for tt in range(tpb):
                    t = g * tpb + tt
                    nc.tensor.matmul(
                        sp,
                        lhsT=tenth_m,
                        rhs=u_hat[:, t, :],
                        start=(tt == 0),
                        stop=(tt == tpb - 1),
                    )
                pick_copy_eng(SA_COST)(s_all[:, g, :], sp)
            squash_group(gr, last=False)

        def rt12_a(gr, i):
            g0, gb = GROUPS[gr]
            tg = gb * tpb
            gs = slice(g0, g0 + gb)
            ts = slice(g0 * tpb, (g0 + gb) * tpb)
            # b update: b (+)= sum_d u_hat * v_prev  (bf16 2x tree adds)
            vb = (
                vrep_bf[:, gs, :]
                .broadcast_to([128, gb, CD, tpb])
                .rearrange("p g c t -> p g t c")
            )
            tail_eng = nc.gpsimd if (GP_ROUTING and gr >= NG - 2) else nc.vector
            tail_eng.tensor_mul(w4[:, gs, :, :], u4[:, gs, :, :], vb)
            wg = w5[:, ts, :, :]
            tail_eng.tensor_add(wg[:, :, :, 0:8], wg[:, :, :, 0:8], wg[:, :, :, 8:16])
            tail_eng.tensor_add(wg[:, :, :, 0:4], wg[:, :, :, 0:4], wg[:, :, :, 4:8])
            tail_eng.tensor_add(wg[:, :, :, 0:2], wg[:, :, :, 0:2], wg[:, :, :, 2:4])
            bg = b3[:, ts, :]
            badd_eng = nc.gpsimd if GP_ROUTING else nc.vector
            if i == 1:
                badd_eng.tensor_add(bg, wg[:, :, :, 0], wg[:, :, :, 1])
            else:
                badd_eng.tensor_add(wg[:, :, :, 0], wg[:, :, :, 0], wg[:, :, :, 1])
                badd_eng.tensor_add(bg, bg, wg[:, :, :, 0])
            track("v", 1500 * gb)  # w4 mul + tree adds + recip
            # softmax over capsules (no max-subtraction: |b| is small)
            fs = slice(g0 * tpb * C, (g0 + gb) * tpb * C)
            e_g = e_exp[:, fs]
            track("s", 450)
            nc.scalar.activation(e_g, b_log[:, fs], mybir.ActivationFunctionType.Exp)
            ss_g = ssum[:, ts]
            nc.vector.tensor_reduce(
                ss_g,
                e_g.rearrange("p (t c) -> p t c", c=C),
                axis=mybir.AxisListType.X,
                op=mybir.AluOpType.add,
            )
            nc.vector.reciprocal(ss_g, ss_g)
            rb = ss_g.broadcast_to([128, tg, C])
            c_g = c_sm[:, fs]
            cmul_eng = nc.gpsimd if GP_ROUTING else nc.vector
            cmul_eng.tensor_mul(
                c_g.rearrange("p (t c) -> p t c", c=C),
                e_g.rearrange("p (t c) -> p t c", c=C),
                rb,
            )

        def rt12_b(gr, i):
            # s[c,d] = sum_n c*u via per-tile matmuls with c stationary
            g0, gb = GROUPS[gr]
            for g in range(g0, g0 + gb):
                pp = ps_P.tile([C, CD], F32, tag="psP")
                for tt in range(tpb):
                    t = g * tpb + tt
                    nc.tensor.matmul(
                        pp,
                        lhsT=c_sm[:, t * C : (t + 1) * C],
                        rhs=u_hat[:, t, :],
                        start=(tt == 0),
                        stop=(tt == tpb - 1),
                    )
                pm = pool_sm.tile([C, CD], BF16, tag="pm")
                nc.vector.tensor_mul(pm, pp, mask)
                sp = ps_S.tile([128, CD], F32, tag="psS")
                nc.tensor.matmul(
                    sp, lhsT=ones_m[0:C, :], rhs=pm, start=True, stop=True
                )
                pick_copy_eng(SA_COST)(s_all[:, g, :], sp)
            squash_group(gr, last=(i == 2))

        # ---------------- interleaved emission ----------------
        # 2-slot lag: iteration 1 of group g-1 and iteration 2 of group g-2
        # run during group g's phase-1.  (A tighter 1-slot lag serializes the
        # within-slot chain and head-of-line-blocks the in-order queues.)
        pending = []
        for gr in range(NG):
            g0, gb = GROUPS[gr]
            for t in range(g0 * tpb, (g0 + gb) * tpb):
                done = emit_trans(t)
                if done:
                    for tp, xv in pending:
                        emit_gemm(tp, xv)
                    pending = done
            # group end: flush a half pair, then drain all pending GEMMs so
            # this group's u_hat is fully emitted before its routing reads it
            for tp, xv in pending + flush_pair():
                emit_gemm(tp, xv)
            pending = []
            if gr >= 1:
                rt12_a(gr - 1, 1)
            if gr >= 2:
                rt12_a(gr - 2, 2)
            rt0(gr)
            if gr >= 1:
                rt12_b(gr - 1, 1)
            if gr >= 2:
                rt12_b(gr - 2, 2)
        # tail: batch the independent a-stages so their chains overlap across
        # engines instead of head-of-line blocking the in-order queues.
        rt12_a(NG - 1, 1)
        rt12_b(NG - 1, 1)
        if NG >= 2:
            rt12_a(NG - 2, 2)
        rt12_a(NG - 1, 2)
        if NG >= 2:
            rt12_b(NG - 2, 2)
        rt12_b(NG - 1, 2)


# ----------------------------------------------------------------------------
_NC_CACHE = {}


def _get_nc():
    key = (BSH, N)
    if key not in _NC_CACHE:
        _NC_CACHE[key] = build_core_program()
    return _NC_CACHE[key]


def _run(x, W, **kw):
    from concourse.bass_utils import run_bass_kernel_spmd

    import ml_dtypes

    nc = _get_nc()
    x = np.ascontiguousarray(x, dtype=np.float32)
    W = np.ascontiguousarray(W, dtype=np.float32)
    ident = np.eye(128, dtype=ml_dtypes.bfloat16)
    mask = np.kron(np.eye(C, dtype=np.float32), np.ones((1, D), np.float32)).astype(
        ml_dtypes.bfloat16
    )
    shards = x.reshape(NCORES, BSH * N, K)
    in_maps = [
        {"x": shards[c], "W": W, "ident": ident, "mask": mask} for c in range(NCORES)
    ]
    res = run_bass_kernel_spmd(nc, in_maps, core_ids=list(range(NCORES)), **kw)
    v = np.concatenate(
        [res.results[c]["v"].reshape(BSH, C, D) for c in range(NCORES)], axis=0
    )
    return v, res


def kernel(x, W):
    v, _ = _run(x, W)
    return v


def kernel_timed(x, W):
    v, res = _run(x, W, trace=True)
    return v, res.exec_time_ns


def kernel_traced(x, W):
    v, res = _run(x, W, trace=True)
    return v, res



# revision 30
# speedup vs baseline: 1.1147x; 1.1147x over previous
"""Trainium2 Bass kernel for CapsuleLayer (dynamic routing), 8-core data-parallel.

Problem: x [128, 1152, 512] f32, W [512, 160] f32.
  u_hat = (x @ W).reshape(B, N, 10, 16)
  b = 0; 3 routing iterations of softmax/weighted-sum/squash.
Output: v [128, 10, 16] f32.

Sharding: data-parallel over batch. Each of the 8 cores gets 16 batches
(x shard [16*1152, 512]) and the full W; no cross-core communication.

Per-core pipeline:
  Phase 1 (streamed over 144 row-tiles of 128):
    - DMA x tile f32 -> SBUF, cast to bf16 (DVE or SWDGE cast-DMA)
    - transpose each [128,128] block on the PE as a *regular* bf16 matmul
      against an identity moving operand (full speed, unlike transpose-mode)
    - u_hat tile = xT.T @ W accumulated over the 4 k-chunks in PSUM,
      copied to SBUF as bf16 in [n, 160] layout (natural for routing)
  Phase 2 (on-chip routing, u_hat resident in SBUF):
    - capsule sums s: wide DVE tree-adds over the 9 tiles of each batch,
      then a single ones-column matmul for the 128-partition reduction
    - softmax over capsules without max-subtraction (|b| stays small)
    - squash per batch on [1,160] tiles; b update via one whole-core
      multiply + grouped reduce
"""

import os
import sys

import numpy as np

sys.path.insert(0, "/opt/trn_rl_repo")

import concourse.bass as bass
import concourse.tile as tile
import concourse.mybir as mybir
from concourse import bacc

F32 = mybir.dt.float32
BF16 = mybir.dt.bfloat16

B, N, K, C, D = 128, 1152, 512, 10, 16
CD = C * D  # 160
NCORES = 8
BSH = B // NCORES  # batches per core

XB_BATCH = 4  # n-tiles per x-load DMA (steady state)
GP_ROUTING = True  # offload f32 (1x-anyway) routing elementwise work to gpsimd
RSQRT_NEWTON = 1  # >0: squash sqrt via quake-rsqrt, entirely on DVE (tiny ops);
#                   keeps ACT on the exp table set all kernel (no table reloads)
# greedy psum->sbuf copy balancing: estimated ns per copy kind per engine.
# gpsimd cannot access PSUM, so only v (DVE) and s (ACT) are eligible.
# DVE reads bf16-PSUM at 2x, ACT only at 1x; both are 1x on f32-PSUM.
XT2_COST = {"v": 658, "s": 996}  # paired 2-tile bf16 xT copy (1024 elem)
XT1_COST = {"v": 392, "s": 570}  # singleton xT copy at a group boundary
UH_COST = {"v": 625, "s": 585}
SA_COST = {"v": 292, "s": 318}


def build_core_program(bsh=BSH, n_len=N, nc=None):
    """Build the single-core Bass program for a shard of `bsh` batches."""
    assert n_len % 128 == 0
    tpb = n_len // 128  # row-tiles per batch
    nt = bsh * tpb  # total row-tiles
    rows = bsh * n_len

    if nc is None:
        nc = bacc.Bacc("TRN2", target_bir_lowering=False, debug=False)

    x_in = nc.declare_dram_parameter("x", [rows, K], F32, isOutput=False).ap()
    w_in = nc.declare_dram_parameter("W", [K, CD], F32, isOutput=False).ap()
    id_in = nc.declare_dram_parameter("ident", [128, 128], BF16, isOutput=False).ap()
    mk_in = nc.declare_dram_parameter("mask", [C, CD], BF16, isOutput=False).ap()
    v_out = nc.declare_dram_parameter("v", [bsh, CD], F32, isOutput=True).ap()

    with tile.TileContext(nc) as tc:
        _build_body(tc, x_in, w_in, id_in, mk_in, v_out, bsh, tpb)
    nc.finalize()
    return nc


def _build_body(tc, x_in, w_in, id_in, mk_in, v_out, bsh, tpb):
    nc = tc.nc
    nt = bsh * tpb
    KT = K // 128  # 4 contraction chunks

    from contextlib import ExitStack

    with ExitStack() as ctx:
        singles = ctx.enter_context(tc.tile_pool(name="singles", bufs=1))
        persist = ctx.enter_context(tc.tile_pool(name="persist", bufs=1))
        pool_xb = ctx.enter_context(tc.tile_pool(name="xb", bufs=4))
        pool_xT = ctx.enter_context(tc.tile_pool(name="xT", bufs=8))
        pool_sm = ctx.enter_context(tc.tile_pool(name="smalls", bufs=6))
        ps_U = ctx.enter_context(tc.tile_pool(name="psU", bufs=2, space="PSUM"))
        ps_P = ctx.enter_context(tc.tile_pool(name="psP", bufs=2, space="PSUM"))
        ps_S = ctx.enter_context(tc.tile_pool(name="psS", bufs=2, space="PSUM"))
        ps_T = ctx.enter_context(tc.tile_pool(name="psT", bufs=2, space="PSUM"))

        # --- constants ---
        ident = singles.tile([128, 128], BF16)
        nc.sync.dma_start(out=ident, in_=id_in)
        mask = singles.tile([C, CD], BF16)
        nc.sync.dma_start(out=mask, in_=mk_in)
        # all-ones / all-0.1 stationary operands: the capsule-sum matmuls use
        # M=128 so the (identical) column sums land replicated on all
        # partitions, which lets squash and the b-update run without any
        # partition-broadcast (illegal on DVE).
        ones_m = singles.tile([128, 128], BF16)
        nc.vector.memset(ones_m, 1.0)
        tenth_m = singles.tile([128, 128], BF16)
        nc.vector.memset(tenth_m, 1.0 / C)

        w_f32 = singles.tile([128, KT, CD], F32)
        nc.sync.dma_start(out=w_f32, in_=w_in.rearrange("(j p) c -> p j c", p=128))
        w_bf = singles.tile([128, KT, CD], BF16)
        nc.vector.tensor_copy(w_bf, w_f32)

        # --- persistent tensors ---
        u_hat = persist.tile([128, nt, CD], BF16)
        w_scr = persist.tile([128, nt, CD], BF16)
        b_log = persist.tile([128, nt * C], F32)
        e_exp = persist.tile([128, nt * C], F32)
        c_sm = persist.tile([128, nt * C], BF16)
        ssum = persist.tile([128, nt], F32)
        s_all = persist.tile([128, bsh, CD], F32)
        sq_all = persist.tile([128, bsh, CD], F32)
        n2_all = persist.tile([128, bsh * C], F32)
        fc_all = persist.tile([128, bsh * C], F32)
        y_all = persist.tile([128, bsh * C], F32)
        t_all = persist.tile([128, bsh * C], F32)
        vrep_bf = persist.tile([128, bsh, CD], BF16)
        vrep = persist.tile([128, bsh, CD], F32)

        # views
        u4 = u_hat[:].rearrange("p (g t) c -> p g t c", g=bsh)
        w4 = w_scr[:].rearrange("p (g t) c -> p g t c", g=bsh)
        w5 = w_scr[:].rearrange("p t (c d) -> p t c d", d=D)
        b3 = b_log[:].rearrange("p (t c) -> p t c", c=C)

        # ---------------- Phase 1 emitters ----------------
        # Transposes run on the PE in transpose-mode (bf16 out in PSUM);
        # TWO tiles share one PSUM bank and get ONE int32-bitcast copy
        # (halves ACT element count — ACT gets no 2x accel on bf16-PSUM —
        # and halves the fixed per-op cost).  The GEMMs of pair m-1 are
        # emitted after the transposes of pair m so the PE never stalls on
        # the copy.
        I32 = mybir.dt.int32
        # x-load batches: tiny first batches so the PE starts early.
        XBATCHES = []
        _t = 0
        _sizes = [2, 2] + [XB_BATCH] * nt
        for sz in _sizes:
            if _t >= nt:
                break
            sz = min(sz, nt - _t)
            XBATCHES.append((_t, sz))
            _t += sz
        TILE2XB = {}
        for _bi, (_ts0, _tn) in enumerate(XBATCHES):
            for _off in range(_tn):
                TILE2XB[_ts0 + _off] = (_bi, _off)

        xb_cur = [None]
        pu_cur = [None]
        copy_eng = {
            "v": nc.vector.tensor_copy,
            "s": nc.scalar.copy,
            "g": nc.gpsimd.tensor_copy,
        }
        # running per-engine busy-ns estimates; phase-2 emitters call track()
        # so copy placement adapts to where the schedule actually is.
        load = {"v": 0.0, "s": 0.0}

        def track(e, ns):
            load[e] += ns

        def pick_copy_eng(cost):
            e = min("vs", key=lambda k: load[k] + cost[k])
            load[e] += cost[e]
            return copy_eng[e]

        pair_state = {"pt8": None, "tiles": []}

        def flush_pair():
            pt8 = pair_state["pt8"]
            tiles = pair_state["tiles"]
            if pt8 is None:
                return []
            xt8 = pool_xT.tile([128, 2, KT, 128], BF16, tag="xt8")
            if len(tiles) == 2:
                pick_copy_eng(XT2_COST)(xt8, pt8)
            else:
                pick_copy_eng(XT1_COST)(xt8[:, 0:1], pt8[:, 0:1])
            out = [(t, xt8[:, i]) for i, t in enumerate(tiles)]
            pair_state["pt8"] = None
            pair_state["tiles"] = []
            return out

        def emit_trans(t):
            tb, tt = TILE2XB[t]
            if tt == 0:
                ts0, tn = XBATCHES[tb]
                xb = pool_xb.tile([128, tn, K], BF16, tag=f"xb{tn}")
                src = x_in[ts0 * 128 : (ts0 + tn) * 128, :]
                nc.gpsimd.dma_start(
                    out=xb, in_=src.rearrange("(t p) k -> p t k", p=128)
                )
                xb_cur[0] = xb
            xb = xb_cur[0]
            if pair_state["pt8"] is None:
                pair_state["pt8"] = ps_T.tile([128, 2, KT, 128], BF16, tag="psT8", name="pt8")
            pt8 = pair_state["pt8"]
            sl = len(pair_state["tiles"])
            for j in range(KT):
                nc.tensor.matmul(
                    pt8[:, sl, j, :],
                    lhsT=xb[:, tt, j * 128 : (j + 1) * 128],
                    rhs=ident,
                    start=True,
                    stop=True,
                    is_transpose=True,
                )
            pair_state["tiles"].append(t)
            if len(pair_state["tiles"]) == 2:
                return flush_pair()
            return []

        UCP = 3 if tpb % 3 == 0 else (2 if tpb % 2 == 0 else 1)  # tiles per u-copy

        def emit_gemm(t, xt4):
            lt = t % tpb
            loc = lt % UCP
            if loc == 0 or pu_cur[0] is None:
                pu_cur[0] = ps_U.tile([128, UCP, CD], F32, tag="psU2", name="pu2")
            pu2 = pu_cur[0]
            for j in range(KT):
                nc.tensor.matmul(
                    pu2[:, loc, :],
                    lhsT=xt4[:, j, :],
                    rhs=w_bf[:, j, :],
                    start=(j == 0),
                    stop=(j == KT - 1),
                )
            if loc == UCP - 1:
                pick_copy_eng(UH_COST)(u_hat[:, t - UCP + 1 : t + 1, :], pu2)
                pu_cur[0] = None
            elif lt == tpb - 1:
                pick_copy_eng(UH_COST)(
                    u_hat[:, t - loc : t + 1, :], pu2[:, 0 : loc + 1, :]
                )
                pu_cur[0] = None

        # ---------------- Phase 2 (routing) emitters, per group ----------
        # Non-uniform groups: big groups early (throughput — fixed per-op
        # costs amortize while phase-1 hides them), small groups last (the
        # post-phase-1 tail is latency-bound on the serial per-group chain,
        # which scales with group size).
        if bsh == 16:
            GSIZES = [3, 3, 2, 2, 2, 2, 1, 1]
        else:
            _ng = 8 if bsh % 8 == 0 else (4 if bsh % 4 == 0 else bsh)
            GSIZES = [bsh // _ng] * _ng
        GROUPS = []
        _o = 0
        for _s in GSIZES:
            GROUPS.append((_o, _s))
            _o += _s
        assert _o == bsh
        NG = len(GROUPS)

        def squash_group(gr, last):
            # v = s * sqrt(n2)/(1+n2)  (the +1e-7 of the reference only
            # perturbs v by O(1e-7) absolute, so it is dropped)
            g0, gb = GROUPS[gr]
            gs = slice(g0, g0 + gb)
            cs = slice(g0 * C, (g0 + gb) * C)
            s_g = s_all[:, gs, :]
            sq_g = sq_all[:, gs, :]
            n2_g = n2_all[:, cs]
            fc_g = fc_all[:, cs]
            track("s", 450)
            nc.scalar.square(sq_g, s_g)
            nc.vector.tensor_reduce(
                n2_g,
                sq_g.rearrange("p g (c d) -> p (g c) d", d=D),
                axis=mybir.AxisListType.X,
                op=mybir.AluOpType.add,
            )
            nc.vector.tensor_scalar_add(fc_g, n2_g, 1.0)
            nc.vector.reciprocal(fc_g, fc_g)
            if RSQRT_NEWTON > 0:
                # quake rsqrt: DVE int seed, gpsimd newton.  Keeps the ACT
                # engine on the exp table set for the whole kernel.
                I32 = mybir.dt.int32
                y_g = y_all[:, cs]
                t_g = t_all[:, cs]
                nc.vector.tensor_scalar(
                    out=y_g.bitcast(I32),
                    in0=n2_g.bitcast(I32),
                    scalar1=1,
                    scalar2=None,
                    op0=mybir.AluOpType.logical_shift_right,
                )
                nc.vector.tensor_scalar(
                    out=y_g.bitcast(I32),
                    in0=y_g.bitcast(I32),
                    scalar1=-1,
                    scalar2=0x5F3759DF,
                    op0=mybir.AluOpType.mult,
                    op1=mybir.AluOpType.add,
                )
                for _ in range(RSQRT_NEWTON):
                    nc.vector.tensor_mul(t_g, y_g, y_g)
                    nc.vector.tensor_mul(t_g, t_g, n2_g)
                    nc.vector.tensor_scalar(
                        out=t_g,
                        in0=t_g,
                        scalar1=-0.5,
                        scalar2=1.5,
                        op0=mybir.AluOpType.mult,
                        op1=mybir.AluOpType.add,
                    )
                    nc.vector.tensor_mul(y_g, y_g, t_g)
                nc.vector.tensor_mul(y_g, y_g, n2_g)  # sqrt(n2) = n2*rsqrt
                nc.vector.tensor_mul(fc_g, fc_g, y_g)  # sqrt(n2)/(1+n2)
            else:
                nc.scalar.sqrt(n2_g, n2_g)
                nc.vector.tensor_mul(fc_g, fc_g, n2_g)  # sqrt(n2)/(1+n2)
            track("v", 800 * gb)
            fb = fc_g.broadcast_to([128, gb * C, D])
            out = (vrep if last else vrep_bf)[:, gs, :]
            out_eng = nc.gpsimd if GP_ROUTING else nc.vector
            out_eng.tensor_mul(
                out.rearrange("p g (c d) -> p (g c) d", d=D),
                s_g.rearrange("p g (c d) -> p (g c) d", d=D),
                fb,
            )
            if last:
                nc.sync.dma_start(
                    out=v_out[g0 : g0 + gb, :], in_=vrep[0:1, gs, :]
                )

        def rt0(gr):
            # s0 = 0.1 * sum_{n,t} u_hat per batch, on the PE
            g0, gb = GROUPS[gr]
            for g in range(g0, g0 + gb):
                sp = ps_S.tile([128, CD], F32, tag="psS")
                for tt in range(tpb):
                    t = g * tpb + tt
                    nc.tensor.matmul(
                        sp,
                        lhsT=tenth_m,
                        rhs=u_hat[:, t, :],
                        start=(tt == 0),
                        stop=(tt == tpb - 1),
                    )
                pick_copy_eng(SA_COST)(s_all[:, g, :], sp)
            squash_group(gr, last=False)

        def rt12_a(gr, i):
            g0, gb = GROUPS[gr]
            tg = gb * tpb
            gs = slice(g0, g0 + gb)
            ts = slice(g0 * tpb, (g0 + gb) * tpb)
            # b update: b (+)= sum_d u_hat * v_prev  (bf16 2x tree adds)
            vb = (
                vrep_bf[:, gs, :]
                .broadcast_to([128, gb, CD, tpb])
                .rearrange("p g c t -> p g t c")
            )
            nc.vector.tensor_mul(w4[:, gs, :, :], u4[:, gs, :, :], vb)
            wg = w5[:, ts, :, :]
            nc.vector.tensor_add(wg[:, :, :, 0:8], wg[:, :, :, 0:8], wg[:, :, :, 8:16])
            nc.vector.tensor_add(wg[:, :, :, 0:4], wg[:, :, :, 0:4], wg[:, :, :, 4:8])
            nc.vector.tensor_add(wg[:, :, :, 0:2], wg[:, :, :, 0:2], wg[:, :, :, 2:4])
            bg = b3[:, ts, :]
            badd_eng = nc.gpsimd if GP_ROUTING else nc.vector
            if i == 1:
                badd_eng.tensor_add(bg, wg[:, :, :, 0], wg[:, :, :, 1])
            else:
                badd_eng.tensor_add(wg[:, :, :, 0], wg[:, :, :, 0], wg[:, :, :, 1])
                badd_eng.tensor_add(bg, bg, wg[:, :, :, 0])
            track("v", 1500 * gb)  # w4 mul + tree adds + recip
            # softmax over capsules (no max-subtraction: |b| is small)
            fs = slice(g0 * tpb * C, (g0 + gb) * tpb * C)
            e_g = e_exp[:, fs]
            track("s", 450)
            nc.scalar.activation(e_g, b_log[:, fs], mybir.ActivationFunctionType.Exp)
            ss_g = ssum[:, ts]
            nc.vector.tensor_reduce(
                ss_g,
                e_g.rearrange("p (t c) -> p t c", c=C),
                axis=mybir.AxisListType.X,
                op=mybir.AluOpType.add,
            )
            nc.vector.reciprocal(ss_g, ss_g)
            rb = ss_g.broadcast_to([128, tg, C])
            c_g = c_sm[:, fs]
            cmul_eng = nc.gpsimd if GP_ROUTING else nc.vector
            cmul_eng.tensor_mul(
                c_g.rearrange("p (t c) -> p t c", c=C),
                e_g.rearrange("p (t c) -> p t c", c=C),
                rb,
            )

        def rt12_b(gr, i):
            # s[c,d] = sum_n c*u via per-tile matmuls with c stationary
            g0, gb = GROUPS[gr]
            for g in range(g0, g0 + gb):
                pp = ps_P.tile([C, CD], F32, tag="psP")
                for tt in range(tpb):
                    t = g * tpb + tt
                    nc.tensor.matmul(
                        pp,
                        lhsT=c_sm[:, t * C : (t + 1) * C],
                        rhs=u_hat[:, t, :],
                        start=(tt == 0),
                        stop=(tt == tpb - 1),
                    )
                pm = pool_sm.tile([C, CD], BF16, tag="pm")
                nc.vector.tensor_mul(pm, pp, mask)
                sp = ps_S.tile([128, CD], F32, tag="psS")
                nc.tensor.matmul(
                    sp, lhsT=ones_m[0:C, :], rhs=pm, start=True, stop=True
                )
                pick_copy_eng(SA_COST)(s_all[:, g, :], sp)
            squash_group(gr, last=(i == 2))

        # ---------------- interleaved emission ----------------
        # 2-slot lag: iteration 1 of group g-1 and iteration 2 of group g-2
        # run during group g's phase-1.  (A tighter 1-slot lag serializes the
        # within-slot chain and head-of-line-blocks the in-order queues.)
        pending = []
        for gr in range(NG):
            g0, gb = GROUPS[gr]
            for t in range(g0 * tpb, (g0 + gb) * tpb):
                done = emit_trans(t)
                if done:
                    for tp, xv in pending:
                        emit_gemm(tp, xv)
                    pending = done
            # group end: flush a half pair, then drain all pending GEMMs so
            # this group's u_hat is fully emitted before its routing reads it
            for tp, xv in pending + flush_pair():
                emit_gemm(tp, xv)
            pending = []
            if gr >= 1:
                rt12_a(gr - 1, 1)
            if gr >= 2:
                rt12_a(gr - 2, 2)
            rt0(gr)
            if gr >= 1:
                rt12_b(gr - 1, 1)
            if gr >= 2:
                rt12_b(gr - 2, 2)
        # tail: batch the independent a-stages so their chains overlap across
        # engines instead of head-of-line blocking the in-order queues.
        rt12_a(NG - 1, 1)
        rt12_b(NG - 1, 1)
        if NG >= 2:
            rt12_a(NG - 2, 2)
        rt12_a(NG - 1, 2)
        if NG >= 2:
            rt12_b(NG - 2, 2)
        rt12_b(NG - 1, 2)


# ----------------------------------------------------------------------------
_NC_CACHE = {}


def _get_nc():
    key = (BSH, N)
    if key not in _NC_CACHE:
        _NC_CACHE[key] = build_core_program()
    return _NC_CACHE[key]


def _run(x, W, **kw):
    from concourse.bass_utils import run_bass_kernel_spmd

    import ml_dtypes

    nc = _get_nc()
    x = np.ascontiguousarray(x, dtype=np.float32)
    W = np.ascontiguousarray(W, dtype=np.float32)
    ident = np.eye(128, dtype=ml_dtypes.bfloat16)
    mask = np.kron(np.eye(C, dtype=np.float32), np.ones((1, D), np.float32)).astype(
        ml_dtypes.bfloat16
    )
    shards = x.reshape(NCORES, BSH * N, K)
    in_maps = [
        {"x": shards[c], "W": W, "ident": ident, "mask": mask} for c in range(NCORES)
    ]
    res = run_bass_kernel_spmd(nc, in_maps, core_ids=list(range(NCORES)), **kw)
    v = np.concatenate(
        [res.results[c]["v"].reshape(BSH, C, D) for c in range(NCORES)], axis=0
    )
    return v, res


def kernel(x, W):
    v, _ = _run(x, W)
    return v


def kernel_timed(x, W):
    v, res = _run(x, W, trace=True)
    return v, res.exec_time_ns


def kernel_traced(x, W):
    v, res = _run(x, W, trace=True)
    return v, res

